# revision 2
# baseline (speedup 1.0000x reference)
"""Causal multi-head self-attention on 8 TRN2 NeuronCores — v2.

Sharding: batch (2) x head-group (4 heads = 256 features) -> 8 cores.

Design (driven by microbenchmarks, see mb*.py):
  - PSUM-port bandwidth is the binding resource in attention (scores write
    + exp read + PV write per row). Everything else follows:
  - all matmul operands bf16 (host-cast); x and W are transposed on the
    HOST so the kernel has zero layout transposes (x.T / W.T DMA'd
    straight into [128, cc, *] SBUF layouts).
  - scores for a head PAIR go into one [128, 2, 512] PSUM tile (2 banks);
    ONE exp instruction covers both heads (halves ACT instruction count).
  - causal diag masking is multiplicative on the exp OUTPUT (bf16, DVE)
    -- no PSUM mask traffic, no extra ACT work.
  - row sums via a ones-column appended to V (65-wide PV stationary);
    z normalized after a small bf16 PE transpose per 128-query block.
  - schedule: proj(g+1) fat units interleaved into attn(g) rounds;
    z-finalize of pair 1 interleaved into pair 2's rounds.
"""

import sys

import numpy as np

sys.path.insert(0, "/opt/trn_rl_repo")

import concourse.bass as bass
import concourse.tile as tile
from concourse import bacc, mybir
from concourse.bass_utils import run_bass_kernel_spmd

B, S, D, H, DK = 2, 2048, 1024, 16, 64
NCORES = 8
HD = 256  # features per core
NHC = 4  # heads per core
NCC = D // 128  # 8 contraction chunks
NG = S // 512  # 4 query groups

f32 = mybir.dt.float32
bf16 = mybir.dt.bfloat16
AF = mybir.ActivationFunctionType
PSUM = bass.MemorySpace.PSUM


def _body(nc, tc, xt, wqt, wkt, wvt, bq2, bk2, bv16, ones16, tri2, id16, out):
    with (
        tc.tile_pool(name="persist", bufs=1) as persist,
        tc.tile_pool(name="u", bufs=6) as u_pool,
        tc.tile_pool(name="zc", bufs=4) as zc_pool,
        tc.tile_pool(name="small", bufs=4) as small,
        tc.tile_pool(name="sp", bufs=2, space=PSUM) as sp_pool,
        tc.tile_pool(name="zp", bufs=2, space=PSUM) as zp_pool,
        tc.tile_pool(name="pp", bufs=2, space=PSUM) as pp_pool,
    ):
        # ---- constants ----
        id16_sb = persist.tile([128, 128], bf16)
        nc.sync.dma_start(out=id16_sb[:], in_=id16)
        tri_sb = persist.tile([128, 2, 128], bf16)
        nc.sync.dma_start(out=tri_sb[:], in_=tri2)
        ones_sb = persist.tile([1, 128], bf16)
        nc.sync.dma_start(out=ones_sb[:], in_=ones16)
        bv_sb = persist.tile([1, HD], bf16)
        nc.sync.dma_start(out=bv_sb[:], in_=bv16)
        bq_sb = persist.tile([128, 2], f32)
        nc.sync.dma_start(out=bq_sb[:], in_=bq2)
        bk_sb = persist.tile([128, 2], f32)
        nc.sync.dma_start(out=bk_sb[:], in_=bk2)

        # ---- persistent tensors ----
        wq_sb = persist.tile([128, NCC, HD], bf16)
        wk_sb = persist.tile([128, NCC, HD], bf16)
        wv_sb = persist.tile([128, NCC, HD], bf16)
        xT = persist.tile([128, NCC, S], bf16)
        qT = persist.tile([128, 2, S], bf16)
        kT = persist.tile([128, 2, S], bf16)
        v_aug = persist.tile([128, S // 128, NHC, 65], bf16)
        z_full = persist.tile([128, S // 128, HD], f32)

        # ---- input DMAs, priority order; tile deps gate compute ----
        nc.sync.dma_start(out=wq_sb[:], in_=wqt.rearrange("(c p) h -> p c h", p=128))
        xt_r = xt.rearrange("(c p) s -> p c s", p=128)
        nc.sync.dma_start(out=xT[:, :, 0:512], in_=xt_r[:, :, 0:512])
        nc.sync.dma_start(out=wk_sb[:], in_=wkt.rearrange("(c p) h -> p c h", p=128))
        nc.sync.dma_start(out=wv_sb[:], in_=wvt.rearrange("(c p) h -> p c h", p=128))
        for g in range(1, NG):
            nc.sync.dma_start(
                out=xT[:, :, bass.ts(g, 512)], in_=xt_r[:, :, bass.ts(g, 512)]
            )

        nc.vector.memset(v_aug[:, :, :, 64], 1.0)

        # ---- projection fat units for group g (8 yields) ----
        def gen_proj(g):
            sl = bass.ts(g, 512)
            for w_sb, bias, dstT in ((wq_sb, bq_sb, qT), (wk_sb, bk_sb, kT)):
                for hdc in range(2):
                    pq = pp_pool.tile([128, 512], f32, tag="pp", name="pq")
                    for cc in range(NCC):
                        nc.tensor.matmul(
                            pq[:],
                            lhsT=w_sb[:, cc, bass.ts(hdc, 128)],
                            rhs=xT[:, cc, sl],
                            start=(cc == 0),
                            stop=(cc == NCC - 1),
                        )
                    nc.vector.tensor_scalar_add(
                        dstT[:, hdc, sl], pq[:], bias[:, hdc : hdc + 1]
                    )
                    yield
            for stl in range(4):
                st = g * 4 + stl
                pv = pp_pool.tile([128, HD], f32, tag="pp", name="pv")
                for cc in range(NCC):
                    nc.tensor.matmul(
                        pv[:],
                        lhsT=xT[:, cc, bass.ts(st, 128)],
                        rhs=wv_sb[:, cc, :],
                        start=(cc == 0),
                        stop=False,
                    )
                nc.tensor.matmul(
                    pv[:], lhsT=ones_sb[0:1, :], rhs=bv_sb[0:1, :],
                    start=False, stop=True,
                )
                nc.vector.tensor_copy(
                    v_aug[:, st, :, 0:64],
                    pv[:].rearrange("p (h d) -> p h d", h=NHC),
                )
                yield

        # ---- z-finalize for one head (4 yields) ----
        def gen_zfin(g, h, zc):
            for qt in range(4):
                zt = pp_pool.tile([128, 65], bf16, tag="pp", name="zt")
                nc.tensor.transpose(
                    zt[:], zc[:, bass.ts(qt, 128)], id16_sb[0:65, 0:65]
                )
                r = small.tile([128, 1], f32, tag="r", name="r")
                nc.vector.reciprocal(r[:], zt[:, 64:65])
                nc.vector.tensor_scalar_mul(
                    z_full[:, g * 4 + qt, bass.ts(h, 64)], zt[:, 0:64], r[:]
                )
                yield

        # ---- attention for group g, incl. z-finalize + out DMA ----
        def gen_attn(g):
            nkc = 4 * g + 4
            zfin_q = []  # pending z-finalize generators
            for p in range(2):
                hA, hB = 2 * p, 2 * p + 1
                zpA = zp_pool.tile([128, 512], f32, tag="zp", name="zpA")
                zpB = zp_pool.tile([128, 512], f32, tag="zp", name="zpB")
                prev = None

                def flush(prev, p, zpA, zpB, nkc):
                    kc, u, q0 = prev
                    for ci, (zp, off) in enumerate(((zpA, 0), (zpB, 0))):
                        nc.tensor.matmul(
                            zp[off : off + 65, q0:512],
                            lhsT=v_aug[:, kc, 2 * p + ci, :],
                            rhs=u[:, ci, q0:512],
                            start=(kc == 0),
                            stop=(kc == nkc - 1),
                        )

                for kc in range(nkc):
                    j = kc - 4 * g
                    q0 = max(0, 128 * j)
                    spAB = sp_pool.tile([128, 2, 512], f32, tag="sp", name="sp")
                    for ci, po in ((0, 0), (1, 64)):
                        nc.tensor.matmul(
                            spAB[:, ci, q0:512],
                            lhsT=kT[po : po + 64, p, bass.ts(kc, 128)],
                            rhs=qT[po : po + 64, p, bass.ds(g * 512 + q0, 512 - q0)],
                            start=True,
                            stop=True,
                        )
                    u = u_pool.tile([128, 2, 512], bf16, tag="u", name="u")
                    nc.scalar.activation(
                        u[:, :, q0:512], spAB[:, :, q0:512], AF.Exp, scale=0.125
                    )
                    if j >= 0:
                        nc.vector.tensor_mul(
                            u[:, :, q0 : q0 + 128], u[:, :, q0 : q0 + 128], tri_sb[:]
                        )
                    if prev is not None:
                        flush(prev, p, zpA, zpB, nkc)
                    prev = (kc, u, q0)
                    if zfin_q:
                        if next(zfin_q[0], StopIteration) is StopIteration:
                            zfin_q.pop(0)
                    yield
                flush(prev, p, zpA, zpB, nkc)
                zcA = zc_pool.tile([65, 512], bf16, tag="zc", name="zcA")
                nc.vector.tensor_copy(zcA[:], zpA[0:65, :])
                zcB = zc_pool.tile([65, 512], bf16, tag="zc", name="zcB")
                nc.vector.tensor_copy(zcB[:], zpB[0:65, :])
                zfin_q.append(gen_zfin(g, hA, zcA))
                zfin_q.append(gen_zfin(g, hB, zcB))
                yield
            for zg in zfin_q:
                for _ in zg:
                    pass
            for stl in range(4):
                st = g * 4 + stl
                nc.sync.dma_start(out=out[bass.ts(st, 128), :], in_=z_full[:, st, :])
            yield

        # ---- main schedule ----
        def drain(gen):
            for _ in gen:
                pass

        drain(gen_proj(0))
        for g in range(NG):
            a = gen_attn(g)
            f = gen_proj(g + 1) if g + 1 < NG else iter(())
            n_attn = 2 * (4 * g + 4) + 3
            ratio = 8.0 / n_attn
            acc = 0.0
            for _ in a:
                acc += ratio
                while acc >= 1.0:
                    next(f, None)
                    acc -= 1.0
            drain(f)


def build():
    nc = bacc.Bacc(
        "TRN2", target_bir_lowering=False, debug=False, num_devices=NCORES
    )
    xt = nc.dram_tensor("xt", [D, S], bf16, kind="ExternalInput")
    wqt = nc.dram_tensor("wqt", [D, HD], bf16, kind="ExternalInput")
    wkt = nc.dram_tensor("wkt", [D, HD], bf16, kind="ExternalInput")
    wvt = nc.dram_tensor("wvt", [D, HD], bf16, kind="ExternalInput")
    bq2 = nc.dram_tensor("bq2", [128, 2], f32, kind="ExternalInput")
    bk2 = nc.dram_tensor("bk2", [128, 2], f32, kind="ExternalInput")
    bv16 = nc.dram_tensor("bv16", [1, HD], bf16, kind="ExternalInput")
    ones16 = nc.dram_tensor("ones16", [1, 128], bf16, kind="ExternalInput")
    tri2 = nc.dram_tensor("tri2", [128, 2, 128], bf16, kind="ExternalInput")
    id16 = nc.dram_tensor("id16", [128, 128], bf16, kind="ExternalInput")
    out = nc.dram_tensor("out", [S, HD], f32, kind="ExternalOutput")
    with tile.TileContext(nc) as tc:
        _body(
            nc, tc, xt.ap(), wqt.ap(), wkt.ap(), wvt.ap(),
            bq2.ap(), bk2.ap(), bv16.ap(), ones16.ap(), tri2.ap(), id16.ap(),
            out.ap(),
        )
    nc.compile()
    return nc


_NC_CACHE = None


def _get_nc():
    global _NC_CACHE
    if _NC_CACHE is None:
        _NC_CACHE = build()
    return _NC_CACHE


def make_in_maps(q_input, W_q, b_q, W_k, b_k, W_v, b_v):
    import ml_dtypes

    bf = ml_dtypes.bfloat16
    ii = np.arange(128)
    tri = (ii[None, :] >= ii[:, None]).astype(bf)  # [k, q] valid
    tri2 = np.ascontiguousarray(np.stack([tri, tri], axis=1))  # [128, 2, 128]
    id16 = np.eye(128, dtype=bf)
    ones16 = np.ones((1, 128), dtype=bf)
    in_maps = []
    for c in range(NCORES):
        b = c // 4
        hs = slice((c % 4) * HD, (c % 4 + 1) * HD)
        in_maps.append(
            {
                "xt": np.ascontiguousarray(
                    np.asarray(q_input[b], dtype=np.float32).T.astype(bf)
                ),
                "wqt": np.ascontiguousarray(
                    np.asarray(W_q[hs], dtype=np.float32).T.astype(bf)
                ),
                "wkt": np.ascontiguousarray(
                    np.asarray(W_k[hs], dtype=np.float32).T.astype(bf)
                ),
                "wvt": np.ascontiguousarray(
                    np.asarray(W_v[hs], dtype=np.float32).T.astype(bf)
                ),
                "bq2": np.ascontiguousarray(
                    np.asarray(b_q[hs], dtype=np.float32).reshape(2, 128).T
                ),
                "bk2": np.ascontiguousarray(
                    np.asarray(b_k[hs], dtype=np.float32).reshape(2, 128).T
                ),
                "bv16": np.asarray(b_v[hs], dtype=np.float32).reshape(1, HD).astype(bf),
                "ones16": ones16,
                "tri2": tri2,
                "id16": id16,
            }
        )
    return in_maps


def assemble(results):
    full = np.empty((B, S, D), dtype=np.float32)
    for c in range(NCORES):
        b = c // 4
        hs = slice((c % 4) * HD, (c % 4 + 1) * HD)
        full[b, :, hs] = results[c]["out"]
    return full


def _ensure_ntff_hook():
    try:
        from antenv import axon_hooks  # noqa: F401

        return
    except ImportError:
        pass
    import types

    try:
        from trn_agent_boot.trn_boot import _ntff_profile_via_ctypes

        hook = _ntff_profile_via_ctypes("/opt/axon/libaxon_pjrt.so")
    except Exception:
        hook = None
    mod = types.ModuleType("antenv.axon_hooks")
    mod._hook = hook
    mod.get_axon_ntff_profile_hook = lambda: mod._hook

    def _set(h):
        mod._hook = h

    mod.set_axon_ntff_profile_hook = _set
    sys.modules["antenv.axon_hooks"] = mod
    try:
        import antenv

        antenv.axon_hooks = mod
    except ImportError:
        pass


def run(inputs_dict, trace=False):
    nc = _get_nc()
    if trace:
        _ensure_ntff_hook()
        import concourse.bass_utils as _bu

        _bu.upload_artifacts = lambda d: d
    in_maps = make_in_maps(**{k: np.asarray(v) for k, v in inputs_dict.items()})
    res = run_bass_kernel_spmd(nc, in_maps, core_ids=list(range(NCORES)), trace=trace)
    return assemble(res.results), res


def kernel(**inputs):
    out, _ = run(inputs, trace=False)
    return out
